# revision 33
# baseline (speedup 1.0000x reference)
# Bidirectional LSTM encoder (nn_Encoder) on Trainium2 via Bass/Tile.
#
# Strategy:
#   - The per-timestep recurrent matmul h @ U ([B,512]@[512,2048]) is
#     LDWEIGHTS-bandwidth-bound on the PE: its cost is independent of batch
#     (B=32 << 128), so data-parallelism over batch buys nothing for the
#     recurrence. Instead: one direction per core (core 0 forward, core 1
#     backward), full batch on each, identical SPMD program - the direction
#     lives entirely in the per-core inputs (core 1 gets time-reversed token
#     indices and the backward weights; its outputs are re-reversed on host).
#   - Fully transposed on-chip layout: z^T, h^T, c^T keep the gate/unit dim on
#     partitions, so the gate elementwise runs on all 128 lanes and the h^T
#     produced by the elementwise is directly the moving operand of the next
#     step's matmuls. No transposes inside the loop.
#   - Pre-phase per core: indirect-DMA embedding gather -> PE-transpose ->
#     zx^T = (xe @ W + b)^T staged to HBM in [t][p][m*32+b] layout so the
#     recurrence streams one contiguous [128, 512] tile per step.
#   - Recurrence: per step 64 accumulating matmuls (stationary = U tiles,
#     moving = h^T chunks, N=32), o-gate tiles last and in a separate PSUM
#     bank so the i/f/g elementwise chain overlaps the o-gate matmuls.
import os
import time

import numpy as np
import jax
import jax.extend
from jax.experimental.shard_map import shard_map
from jax.sharding import Mesh, NamedSharding, PartitionSpec

from concourse import bacc, bass2jax, mybir, tile
import concourse.bass as bass
from concourse.bass import ds, ts
from concourse.masks import make_identity

VOCAB, EMB, UNITS, B, S = 32000, 256, 512, 32, 512
H4 = 4 * UNITS            # 2048 gate width
P = 128
MT = H4 // P              # 16 m-tiles over the gate dim
KC = UNITS // P           # 4 k-chunks over the hidden dim
ECH = EMB // P            # 2 k-chunks over the embedding dim
NTOK = B * S              # 16384 tokens
GT = NTOK // P            # 128 gather tiles
NZ = 512                  # token-chunk per zx matmul (16 timesteps x 32 batch)
NB = NTOK // NZ           # 32 n-chunks
UNROLL = 16               # steps per hardware-loop iteration

FP32 = mybir.dt.float32
DT_C = mybir.dt.float16   # matmul operand dtype (weights, xe^T, h^T)
NP_C = np.float16
DT_Z = mybir.dt.float16   # zx^T HBM staging dtype
AF = mybir.ActivationFunctionType

_CACHE = {}
LAST_RESULT = {}


def _build(s_run=None, skip_pre=False, unroll=UNROLL, staggered=False, hints=(), mm_only=False, ew_only=False, no_barrier=False, pre_reps=1, half_chain=False):
    # s_run/skip_pre are benchmarking knobs (shorter recurrence / no pre-phase);
    # the graded path always uses the defaults.
    if s_run is None:
        s_run = S
    nc = bacc.Bacc(
        "TRN2",
        target_bir_lowering=False,
        debug=False,
        enable_asserts=True,
        num_devices=2,
    )
    emb_in = nc.dram_tensor("emb", [VOCAB, EMB], DT_C, kind="ExternalInput")
    idx_in = nc.dram_tensor("idx", [NTOK], mybir.dt.int32, kind="ExternalInput")
    W_in = nc.dram_tensor("W", [EMB, H4], DT_C, kind="ExternalInput")
    U_in = nc.dram_tensor("U", [UNITS, H4], DT_C, kind="ExternalInput")
    b_in = nc.dram_tensor("b", [P, MT], FP32, kind="ExternalInput")
    h0_in = nc.dram_tensor("h0", [P, P], FP32, kind="ExternalInput")
    c0_in = nc.dram_tensor("c0", [P, P], FP32, kind="ExternalInput")
    outT = nc.dram_tensor("outT", [max(S, s_run) * P, P], DT_C, kind="ExternalOutput")
    hT_o = nc.dram_tensor("hT", [P, P], DT_C, kind="ExternalOutput")
    cT_o = nc.dram_tensor("cT", [P, P], FP32, kind="ExternalOutput")
    # zx^T staging: row = t*128 + p (p = unit-within-m-tile), col = m*32 + b.
    # UNROLL extra zero rows absorb the prefetch overrun of the last iteration.
    zx_kind = (
        "ExternalOutput" if os.environ.get("BASS_LSTM_DEBUG_ZX") else "Internal"
    )
    # Block-major zx staging: block (m, nb) holds zs [128, 512] contiguously,
    # so pre-phase writes run at full DMA bandwidth; the recurrence pays the
    # strided gather on its reads, hidden under the per-step matmul time.
    NBR = max(S, s_run) // 16 + 1   # +1 pad block per m-tile for prefetch overrun
    zxS = nc.dram_tensor("zxS", [MT * NBR * P, NZ], DT_Z, kind=zx_kind)

    def phase_a(tc):
        with (
            tc.tile_pool(name="ppA", bufs=1) as ppA,
            tc.tile_pool(name="wpA", bufs=4) as wpA,
            tc.tile_pool(name="psT", bufs=2, space="PSUM") as psT,
            tc.tile_pool(name="psZ", bufs=2, space="PSUM") as psZ,
        ):
            ident = ppA.tile([P, P], DT_C, tag="ident", name="ident")
            make_identity(nc, ident[:])
            Wk = [ppA.tile([P, H4], DT_C, tag=f"Wk{e}", name=f"Wk{e}") for e in range(ECH)]
            for e in range(ECH):
                nc.sync.dma_start(Wk[e][:], W_in[e * P:(e + 1) * P, :])
            b_sb = ppA.tile([P, MT], FP32, tag="b", name="b_sb")
            nc.sync.dma_start(b_sb[:], b_in[:])
            xeT = [ppA.tile([P, NTOK], DT_C, tag=f"xeT{e}", name=f"xeT{e}") for e in range(ECH)]
            for g in range(GT):
                it = wpA.tile([P, 1], mybir.dt.int32, tag="idx", name="it")
                nc.sync.dma_start(it[:], idx_in[g * P:(g + 1) * P, None])
                xg = wpA.tile([P, EMB], DT_C, tag="xe", name="xg")
                nc.gpsimd.indirect_dma_start(
                    out=xg[:],
                    out_offset=None,
                    in_=emb_in[:],
                    in_offset=bass.IndirectOffsetOnAxis(ap=it[:, :1], axis=0),
                )
                for e in range(ECH):
                    tp = psT.tile([P, P], DT_C, tag="tp", name="tp")
                    nc.tensor.transpose(
                        out=tp[:], in_=xg[:, e * P:(e + 1) * P], identity=ident[:]
                    )
                    nc.vector.tensor_copy(xeT[e][:, g * P:(g + 1) * P], tp[:])
            mn_order = (
                [(m, nb) for m in range(MT) for nb in range(NB)]
                if not no_barrier
                else [(m, nb) for nb in range(NB) for m in range(MT)]
            )
            if True:
                for m, nb in mn_order:
                    pz = psZ.tile([P, NZ], FP32, tag="pz", name="pz")
                    for e in range(ECH):
                        nc.tensor.matmul(
                            pz[:],
                            Wk[e][:, m * P:(m + 1) * P],
                            xeT[e][:, nb * NZ:(nb + 1) * NZ],
                            start=(e == 0),
                            stop=(e == ECH - 1),
                        )
                    zs = wpA.tile([P, NZ], DT_Z, tag="zs", name="zs")
                    nc.scalar.activation(
                        zs[:, 0:256], pz[:, 0:256], AF.Identity,
                        bias=b_sb[:, m:m + 1],
                    )
                    nc.vector.tensor_scalar(
                        zs[:, 256:512], pz[:, 256:512], b_sb[:, m:m + 1], None,
                        op0=mybir.AluOpType.add,
                    )
                    nc.sync.dma_start(
                        zxS[(m * NBR + nb) * P:(m * NBR + nb + 1) * P, :], zs[:]
                    )
    with tile.TileContext(nc) as tc:
        # Phase-B persistent state is allocated and loaded up front so the
        # U-weight DMAs and state initialization overlap phase A.
        with (
            tc.tile_pool(name="ppB", bufs=1) as ppB,
            tc.tile_pool(name="psB", bufs=1, space="PSUM") as psB,
        ):
            Uk = [ppB.tile([P, H4], DT_C, tag=f"Uk{k}", name=f"Uk{k}") for k in range(KC)]
            for k in range(KC):
                nc.sync.dma_start(Uk[k][:], U_in[k * P:(k + 1) * P, :])
            h_sb = [ppB.tile([P, P], DT_C, tag=f"h{i}", name=f"h{i}") for i in range(2)]
            c_sb = [ppB.tile([P, P], FP32, tag=f"c{i}", name=f"c{i}") for i in range(2)]
            h0f = ppB.tile([P, P], FP32, tag="h0f", name="h0f")
            nc.sync.dma_start(h0f[:], h0_in[:])
            nc.vector.tensor_copy(h_sb[0][:], h0f[:])
            nc.sync.dma_start(c_sb[0][:], c0_in[:])
            if not skip_pre:
                for _ in range(pre_reps):
                    phase_a(tc)
            if not no_barrier:
                tc.strict_bb_all_engine_barrier()
            # ---------- Phase B: the 512-step recurrence --------------------
            # [r, m, c] view of the block-major staging: r = nb*128 + p.
            zxS_r = zxS[:].rearrange("(m r) c -> r m c", m=MT)
            zxb = [ppB.tile([P, NZ], DT_Z, tag=f"zxb{u}", name=f"zxb{u}") for u in range(unroll)]
            for u in range(unroll):
                nc.sync.dma_start(
                    zxb[u][:].rearrange("p (m b) -> p m b", m=MT),
                    zxS_r[0:P, :, u * 32:(u + 1) * 32],
                )
            ident_B = ppB.tile([P, P], DT_C, tag="identB", name="ident_B")
            make_identity(nc, ident_B[:])
            zb_ifg = [psB.tile([P, 384], FP32, tag=f"zbifg{i}", name=f"zbifg{i}") for i in range(2)]
            zb_o = [psB.tile([P, P], FP32, tag=f"zbo{i}", name=f"zbo{i}") for i in range(2)]
            # sig holds [sigma(i) | sigma(f)] per unit-half: [p, 2, 64]
            sig = [[ppB.tile([P, 2, 64], FP32, tag=f"sig{i}{h}", name=f"sig{i}{h}")
                    for h in range(2)] for i in range(2)]
            gt_ = [ppB.tile([P, P], FP32, tag=f"gt{i}", name=f"gt{i}") for i in range(2)]
            so_ = [ppB.tile([P, P], DT_C, tag=f"so{i}", name=f"so{i}") for i in range(2)]
            tct = [ppB.tile([P, P], DT_C, tag=f"tc{i}", name=f"tc{i}") for i in range(2)]
            ig_ = [ppB.tile([P, P], FP32, tag=f"ig{i}", name=f"ig{i}") for i in range(2)]
            fc_ = [ppB.tile([P, P], FP32, tag=f"fc{i}", name=f"fc{i}") for i in range(2)]

            if mm_only:
                nc.gpsimd.memset(h_sb[1][:], 0.0)
                nc.gpsimd.memset(c_sb[1][:], 0.0)

            def step(t_ap, u, tv_base):
                par, npar = u % 2, (u + 1) % 2
                # Seed both PSUM banks with zx_t via identity-matmuls, then
                # accumulate h @ U on top: activations read PSUM directly.
                nc.tensor.matmul(zb_ifg[par][:], ident_B[:], zxb[u][:, 0:384],
                                 start=True, stop=False)
                nc.tensor.matmul(zb_o[par][:], ident_B[:], zxb[u][:, 384:512],
                                 start=True, stop=False)
                # i/f/g m-tiles for unit-half 0 first, then half 1, o last
                # (separate PSUM bank): each half's c-chain starts while the
                # PE still streams the rest of the step's weights.
                m_order = (
                    [0, 1, 4, 5, 8, 9, 2, 3, 6, 7, 10, 11, 12, 13, 14, 15]
                    if not ew_only else [0, 12]
                )
                for m in m_order:
                    if m < 12:
                        tgt = zb_ifg[par][:, m * 32:(m + 1) * 32]
                    else:
                        tgt = zb_o[par][:, (m - 12) * 32:(m - 11) * 32]
                    for k in range(KC):
                        nc.tensor.matmul(
                            tgt,
                            Uk[k][:, m * P:(m + 1) * P],
                            h_sb[par][:, k * 32:(k + 1) * 32],
                            start=False,
                            stop=(k == KC - 1),
                            skip_group_check=True,
                        )
                if mm_only:
                    return
                if not half_chain:
                    nc.scalar.activation(
                        sig[par][0][:].rearrange("p g c -> p (g c)"),
                        zb_ifg[par][:, 0:128], AF.Sigmoid)
                    nc.scalar.activation(
                        sig[par][1][:].rearrange("p g c -> p (g c)"),
                        zb_ifg[par][:, 128:256], AF.Sigmoid)
                    nc.scalar.activation(gt_[par][:], zb_ifg[par][:, 256:384], AF.Tanh)
                    sigi = sig[par][0][:].rearrange("p g c -> p (g c)")
                    sigf = sig[par][1][:].rearrange("p g c -> p (g c)")
                    nc.vector.tensor_mul(fc_[par][:], sigf, c_sb[par][:])
                    nc.vector.tensor_mul(ig_[par][:], sigi, gt_[par][:])
                    nc.vector.tensor_add(c_sb[npar][:], fc_[par][:], ig_[par][:])
                    nc.scalar.activation(tct[par][:], c_sb[npar][:], AF.Tanh)
                    nc.scalar.activation(so_[par][:], zb_o[par][:], AF.Sigmoid)
                    nc.vector.tensor_mul(h_sb[npar][:], so_[par][:], tct[par][:])
                    nc.sync.dma_start(outT[ts(t_ap, P), :], h_sb[npar][:])
                    nc.sync.dma_start(
                        zxb[u][:].rearrange("p (m b) -> p m b", m=MT),
                        zxS_r[ds((tv_base + unroll) * (P // 16), P), :, u * 32:(u + 1) * 32],
                    )
                    return
                for h in range(2):
                    cs = slice(h * 64, h * 64 + 64)
                    # sigma over [i|f] of this half: psum cols i at h*64,
                    # f at 128 + h*64 - one ACT op via a 2-run AP.
                    zif = (
                        zb_ifg[par][:, 0:256]
                        .rearrange("p (g c) -> p g c", g=2)[:, :, h * 64:h * 64 + 64]
                    )
                    nc.scalar.activation(sig[par][h][:], zif, AF.Sigmoid)
                    nc.scalar.activation(
                        gt_[par][:, cs], zb_ifg[par][:, 256 + h * 64:256 + h * 64 + 64],
                        AF.Tanh,
                    )
                    nc.vector.tensor_mul(
                        fc_[par][:, cs], sig[par][h][:, 1, :], c_sb[par][:, cs]
                    )
                    nc.vector.tensor_mul(
                        ig_[par][:, cs], sig[par][h][:, 0, :], gt_[par][:, cs]
                    )
                    nc.vector.tensor_add(
                        c_sb[npar][:, cs], fc_[par][:, cs], ig_[par][:, cs]
                    )
                    nc.scalar.activation(
                        tct[par][:, cs], c_sb[npar][:, cs], AF.Tanh
                    )
                nc.scalar.activation(so_[par][:], zb_o[par][:], AF.Sigmoid)
                nc.vector.tensor_mul(h_sb[npar][:], so_[par][:], tct[par][:])
                nc.sync.dma_start(outT[ts(t_ap, P), :], h_sb[npar][:])
                nc.sync.dma_start(
                    zxb[u][:].rearrange("p (m b) -> p m b", m=MT),
                    zxS_r[ds((tv_base + unroll) * (P // 16), P), :, u * 32:(u + 1) * 32],
                )

            assert unroll == 16, "block-major zx gather needs unroll=16"
            with tc.For_i(0, s_run, unroll, staggered_reset=staggered,
                          hint_engines=hints) as tv:
                for u in range(unroll):
                    step(tv + u, u, tv)
            nc.sync.dma_start(hT_o[:], h_sb[0][:])
            nc.sync.dma_start(cT_o[:], c_sb[0][:])
    nc.compile()
    return nc


N_CORES = 2


def _make_exec(nc):
    """Jitted shard_map executor over the first N_CORES neuron devices.

    Same mechanism as bass2jax.run_bass_via_pjrt, kept local so the
    executable and device-resident inputs can be reused for timing.
    """
    bass2jax.install_neuronx_cc_hook()
    partition_name = nc.partition_id_tensor.name if nc.partition_id_tensor else None
    in_names, out_names, out_avals = [], [], []
    for alloc in nc.m.functions[0].allocations:
        if not isinstance(alloc, mybir.MemoryLocationSet):
            continue
        name = alloc.memorylocations[0].name
        if alloc.kind == "ExternalInput":
            if name != partition_name:
                in_names.append(name)
        elif alloc.kind == "ExternalOutput":
            shape = tuple(alloc.tensor_shape)
            dtype = mybir.dt.np(alloc.dtype)
            out_names.append(name)
            out_avals.append(jax.core.ShapedArray(shape, dtype))
    n_params = len(in_names)
    all_in_names = in_names + out_names
    if partition_name is not None:
        all_in_names = all_in_names + [partition_name]

    def _body(*args):
        operands = list(args)
        if partition_name is not None:
            operands.append(bass2jax.partition_id_tensor())
        outs = bass2jax._bass_exec_p.bind(
            *operands,
            out_avals=tuple(out_avals),
            in_names=tuple(all_in_names),
            out_names=tuple(out_names),
            lowering_input_output_aliases=(),
            sim_require_finite=True,
            sim_require_nnan=True,
            nc=nc,
        )
        return tuple(outs)

    devices = jax.devices()[:N_CORES]
    mesh = Mesh(np.asarray(devices), ("core",))
    nin = n_params + len(out_names)
    sharded = jax.jit(
        shard_map(
            _body,
            mesh=mesh,
            in_specs=(PartitionSpec("core"),) * nin,
            out_specs=(PartitionSpec("core"),) * len(out_names),
            check_rep=False,
        ),
        keep_unused=True,
    )
    return sharded, mesh, in_names, out_names, out_avals


def _run_spmd(in_maps):
    """Execute on cores 0..N_CORES-1, retrying through a backend reset if
    the device wedges (intermittent NRT_EXEC_UNIT_UNRECOVERABLE on first
    execution); optionally time repeated executions."""
    last_err = None
    for attempt in range(3):
        try:
            return _run_spmd_inner(in_maps)
        except Exception as e:  # noqa: BLE001 - any runtime error warrants retry
            last_err = e
            LAST_RESULT["retry_error"] = repr(e)
            try:
                jax.clear_caches()
                jax.extend.backend.clear_backends()
            except Exception:
                pass
            _CACHE.pop("exec", None)
            _CACHE.pop("nc", None)
            time.sleep(5)
    raise last_err


def _run_spmd_inner(in_maps):
    if "nc" not in _CACHE:
        _CACHE["nc"] = _build()
        _CACHE["exec"] = _make_exec(_CACHE["nc"])
    sharded, mesh, in_names, out_names, out_avals = _CACHE["exec"]
    sh = NamedSharding(mesh, PartitionSpec("core"))
    args = [
        jax.device_put(
            np.concatenate([np.asarray(m[n]) for m in in_maps], axis=0), sh
        )
        for n in in_names
    ]
    args += [
        jax.device_put(
            np.zeros((N_CORES * av.shape[0], *av.shape[1:]), av.dtype), sh
        )
        for av in out_avals
    ]
    outs = sharded(*args)
    jax.block_until_ready(outs)
    n_time = int(os.environ.get("BASS_LSTM_TIME", "0"))
    if n_time > 0:
        # Serial timing (includes per-exec axon roundtrip)...
        times = []
        for _ in range(n_time):
            t0 = time.perf_counter()
            outs2 = sharded(*args)
            jax.block_until_ready(outs2)
            times.append(time.perf_counter() - t0)
        LAST_RESULT["times_ms"] = [t * 1e3 for t in times]
        # ...and pipelined timing: dispatch a burst, block once. Device
        # executions queue back-to-back, amortizing the tunnel roundtrip.
        burst = 16
        t0 = time.perf_counter()
        pending = [sharded(*args) for _ in range(burst)]
        jax.block_until_ready(pending)
        per = (time.perf_counter() - t0) / burst
        LAST_RESULT["pipelined_ms"] = per * 1e3
        LAST_RESULT["exec_time_ns"] = int(per * 1e9)
    results = []
    for c in range(N_CORES):
        results.append(
            {
                n: np.asarray(outs[i]).reshape(N_CORES, *out_avals[i].shape)[c]
                for i, n in enumerate(out_names)
            }
        )
    return results


def _pack_state(v):
    # [B, UNITS] -> [128, 128] with [p, uc*32 + b] = v[b, uc*128 + p]
    return np.ascontiguousarray(
        np.asarray(v, np.float32).reshape(B, KC, P).transpose(2, 1, 0).reshape(P, P)
    )


def _unpack_state(m):
    # inverse of _pack_state
    return np.ascontiguousarray(
        np.asarray(m, np.float32).reshape(P, KC, B).transpose(2, 1, 0).reshape(B, UNITS)
    )


def _unpack_out(o):
    # [S*128, 128] with row t*128+p, col uc*32+b  ->  [B, S, UNITS]
    return np.ascontiguousarray(
        np.asarray(o, np.float32)
        .reshape(S, P, KC, B)
        .transpose(3, 0, 2, 1)
        .reshape(B, S, UNITS)
    )


def kernel(x, fh, fc, bh, bc, emb, Wf, Uf, bf, Wb, Ub, bb):
    x = np.asarray(x)
    emb_h = np.ascontiguousarray(np.asarray(emb, np.float32).astype(NP_C))
    idx_f = np.ascontiguousarray(x.T.astype(np.int32).reshape(-1))        # [S*B] t-major
    idx_b = np.ascontiguousarray(x.T[::-1].astype(np.int32).reshape(-1))  # reversed time

    def prep(Wm, Um, bm, h0, c0, idx):
        return {
            "emb": emb_h,
            "idx": idx,
            "W": np.ascontiguousarray(np.asarray(Wm, np.float32).astype(NP_C)),
            "U": np.ascontiguousarray(np.asarray(Um, np.float32).astype(NP_C)),
            "b": np.ascontiguousarray(
                np.asarray(bm, np.float32).reshape(MT, P).T
            ),
            "h0": _pack_state(h0),
            "c0": _pack_state(c0),
        }

    in_maps = [
        prep(Wf, Uf, bf, fh, fc, idx_f),
        prep(Wb, Ub, bb, bh, bc, idx_b),
    ]
    r0, r1 = _run_spmd(in_maps)
    out_f = _unpack_out(r0["outT"])
    out_b = _unpack_out(r1["outT"])[:, ::-1, :]
    output = np.concatenate([out_f, out_b], axis=-1)
    fhT = _unpack_state(r0["hT"])
    fcT = _unpack_state(r0["cT"])
    bhT = _unpack_state(r1["hT"])
    bcT = _unpack_state(r1["cT"])
    return output, fhT, fcT, bhT, bcT


# revision 34
# speedup vs baseline: 1.7965x; 1.7965x over previous
# Bidirectional LSTM encoder (nn_Encoder) on Trainium2 via Bass/Tile.
#
# Strategy:
#   - The per-timestep recurrent matmul h @ U ([B,512]@[512,2048]) is
#     LDWEIGHTS-bandwidth-bound on the PE: its cost is independent of batch
#     (B=32 << 128), so data-parallelism over batch buys nothing for the
#     recurrence. Instead: one direction per core (core 0 forward, core 1
#     backward), full batch on each, identical SPMD program - the direction
#     lives entirely in the per-core inputs (core 1 gets time-reversed token
#     indices and the backward weights; its outputs are re-reversed on host).
#   - Fully transposed on-chip layout: z^T, h^T, c^T keep the gate/unit dim on
#     partitions, so the gate elementwise runs on all 128 lanes and the h^T
#     produced by the elementwise is directly the moving operand of the next
#     step's matmuls. No transposes inside the loop.
#   - Pre-phase per core: indirect-DMA embedding gather -> PE-transpose ->
#     zx^T = (xe @ W + b)^T staged to HBM in [t][p][m*32+b] layout so the
#     recurrence streams one contiguous [128, 512] tile per step.
#   - Recurrence: per step 64 accumulating matmuls (stationary = U tiles,
#     moving = h^T chunks, N=32), o-gate tiles last and in a separate PSUM
#     bank so the i/f/g elementwise chain overlaps the o-gate matmuls.
import os
import time

import numpy as np
import jax
import jax.extend
from jax.experimental.shard_map import shard_map
from jax.sharding import Mesh, NamedSharding, PartitionSpec

from concourse import bacc, bass2jax, mybir, tile
import concourse.bass as bass
from concourse.bass import ds, ts
from concourse.masks import make_identity

VOCAB, EMB, UNITS, B, S = 32000, 256, 512, 32, 512
H4 = 4 * UNITS            # 2048 gate width
P = 128
MT = H4 // P              # 16 m-tiles over the gate dim
KC = UNITS // P           # 4 k-chunks over the hidden dim
ECH = EMB // P            # 2 k-chunks over the embedding dim
NTOK = B * S              # 16384 tokens
GT = NTOK // P            # 128 gather tiles
NZ = 512                  # token-chunk per zx matmul (16 timesteps x 32 batch)
NB = NTOK // NZ           # 32 n-chunks
UNROLL = 16               # steps per hardware-loop iteration

FP32 = mybir.dt.float32
DT_C = mybir.dt.float16   # matmul operand dtype (weights, xe^T, h^T)
NP_C = np.float16
DT_Z = mybir.dt.float16   # zx^T HBM staging dtype
AF = mybir.ActivationFunctionType

_CACHE = {}
LAST_RESULT = {}


def _build(s_run=None, skip_pre=False, unroll=UNROLL, staggered=False, hints=(), mm_only=False, ew_only=False, no_barrier=False, pre_reps=1, half_chain=False):
    # s_run/skip_pre are benchmarking knobs (shorter recurrence / no pre-phase);
    # the graded path always uses the defaults.
    if s_run is None:
        s_run = S
    nc = bacc.Bacc(
        "TRN2",
        target_bir_lowering=False,
        debug=False,
        enable_asserts=True,
        num_devices=2,
    )
    emb_in = nc.dram_tensor("emb", [VOCAB, EMB], DT_C, kind="ExternalInput")
    idx_in = nc.dram_tensor("idx", [NTOK], mybir.dt.int32, kind="ExternalInput")
    W_in = nc.dram_tensor("W", [EMB, H4], DT_C, kind="ExternalInput")
    U_in = nc.dram_tensor("U", [UNITS, H4], DT_C, kind="ExternalInput")
    b_in = nc.dram_tensor("b", [P, MT], FP32, kind="ExternalInput")
    h0_in = nc.dram_tensor("h0", [P, P], FP32, kind="ExternalInput")
    c0_in = nc.dram_tensor("c0", [P, P], FP32, kind="ExternalInput")
    outT = nc.dram_tensor("outT", [max(S, s_run) * P, P], DT_C, kind="ExternalOutput")
    hT_o = nc.dram_tensor("hT", [P, P], DT_C, kind="ExternalOutput")
    cT_o = nc.dram_tensor("cT", [P, P], FP32, kind="ExternalOutput")
    # zx^T staging: row = t*128 + p (p = unit-within-m-tile), col = m*32 + b.
    # UNROLL extra zero rows absorb the prefetch overrun of the last iteration.
    zx_kind = (
        "ExternalOutput" if os.environ.get("BASS_LSTM_DEBUG_ZX") else "Internal"
    )
    # Block-major zx staging: block (m, nb) holds zs [128, 512] contiguously,
    # so pre-phase writes run at full DMA bandwidth; the recurrence pays the
    # strided gather on its reads, hidden under the per-step matmul time.
    NBR = max(S, s_run) // 16 + 1   # +1 pad block per m-tile for prefetch overrun
    zxS = nc.dram_tensor("zxS", [MT * NBR * P, NZ], DT_Z, kind=zx_kind)

    def phase_a(tc):
        with (
            tc.tile_pool(name="ppA", bufs=1) as ppA,
            tc.tile_pool(name="wpA", bufs=4) as wpA,
            tc.tile_pool(name="psT", bufs=2, space="PSUM") as psT,
            tc.tile_pool(name="psZ", bufs=2, space="PSUM") as psZ,
        ):
            ident = ppA.tile([P, P], DT_C, tag="ident", name="ident")
            make_identity(nc, ident[:])
            Wk = [ppA.tile([P, H4], DT_C, tag=f"Wk{e}", name=f"Wk{e}") for e in range(ECH)]
            for e in range(ECH):
                nc.sync.dma_start(Wk[e][:], W_in[e * P:(e + 1) * P, :])
            b_sb = ppA.tile([P, MT], FP32, tag="b", name="b_sb")
            nc.sync.dma_start(b_sb[:], b_in[:])
            xeT = [ppA.tile([P, NTOK], DT_C, tag=f"xeT{e}", name=f"xeT{e}") for e in range(ECH)]
            for g in range(GT):
                it = wpA.tile([P, 1], mybir.dt.int32, tag="idx", name="it")
                nc.sync.dma_start(it[:], idx_in[g * P:(g + 1) * P, None])
                xg = wpA.tile([P, EMB], DT_C, tag="xe", name="xg")
                nc.gpsimd.indirect_dma_start(
                    out=xg[:],
                    out_offset=None,
                    in_=emb_in[:],
                    in_offset=bass.IndirectOffsetOnAxis(ap=it[:, :1], axis=0),
                )
                for e in range(ECH):
                    tp = psT.tile([P, P], DT_C, tag="tp", name="tp")
                    nc.tensor.transpose(
                        out=tp[:], in_=xg[:, e * P:(e + 1) * P], identity=ident[:]
                    )
                    nc.vector.tensor_copy(xeT[e][:, g * P:(g + 1) * P], tp[:])
            mn_order = (
                [(m, nb) for m in range(MT) for nb in range(NB)]
                if not no_barrier
                else [(m, nb) for nb in range(NB) for m in range(MT)]
            )
            if True:
                for m, nb in mn_order:
                    pz = psZ.tile([P, NZ], FP32, tag="pz", name="pz")
                    for e in range(ECH):
                        nc.tensor.matmul(
                            pz[:],
                            Wk[e][:, m * P:(m + 1) * P],
                            xeT[e][:, nb * NZ:(nb + 1) * NZ],
                            start=(e == 0),
                            stop=(e == ECH - 1),
                        )
                    zs = wpA.tile([P, NZ], DT_Z, tag="zs", name="zs")
                    nc.scalar.activation(
                        zs[:, 0:256], pz[:, 0:256], AF.Identity,
                        bias=b_sb[:, m:m + 1],
                    )
                    nc.vector.tensor_scalar(
                        zs[:, 256:512], pz[:, 256:512], b_sb[:, m:m + 1], None,
                        op0=mybir.AluOpType.add,
                    )
                    nc.sync.dma_start(
                        zxS[(m * NBR + nb) * P:(m * NBR + nb + 1) * P, :], zs[:]
                    )
    with tile.TileContext(nc) as tc:
        # Phase-B persistent state is allocated and loaded up front so the
        # U-weight DMAs and state initialization overlap phase A.
        with (
            tc.tile_pool(name="ppB", bufs=1) as ppB,
            tc.tile_pool(name="psB", bufs=1, space="PSUM") as psB,
        ):
            Uk = [ppB.tile([P, H4], DT_C, tag=f"Uk{k}", name=f"Uk{k}") for k in range(KC)]
            for k in range(KC):
                nc.sync.dma_start(Uk[k][:], U_in[k * P:(k + 1) * P, :])
            h_sb = [ppB.tile([P, P], DT_C, tag=f"h{i}", name=f"h{i}") for i in range(2)]
            c_sb = [ppB.tile([P, P], FP32, tag=f"c{i}", name=f"c{i}") for i in range(2)]
            h0f = ppB.tile([P, P], FP32, tag="h0f", name="h0f")
            nc.sync.dma_start(h0f[:], h0_in[:])
            nc.vector.tensor_copy(h_sb[0][:], h0f[:])
            nc.sync.dma_start(c_sb[0][:], c0_in[:])
            if not skip_pre:
                for _ in range(pre_reps):
                    phase_a(tc)
            if not no_barrier:
                tc.strict_bb_all_engine_barrier()
            # ---------- Phase B: the 512-step recurrence --------------------
            # [r, m, c] view of the block-major staging: r = nb*128 + p.
            zxS_r = zxS[:].rearrange("(m r) c -> r m c", m=MT)
            zxb = [ppB.tile([P, NZ], DT_Z, tag=f"zxb{u}", name=f"zxb{u}") for u in range(unroll)]
            for u in range(unroll):
                nc.sync.dma_start(
                    zxb[u][:].rearrange("p (m b) -> p m b", m=MT),
                    zxS_r[0:P, :, u * 32:(u + 1) * 32],
                )
            ident_B = ppB.tile([P, P], DT_C, tag="identB", name="ident_B")
            make_identity(nc, ident_B[:])
            zb_ifg = [psB.tile([P, 384], FP32, tag=f"zbifg{i}", name=f"zbifg{i}") for i in range(2)]
            zb_o = [psB.tile([P, P], FP32, tag=f"zbo{i}", name=f"zbo{i}") for i in range(2)]
            # sig holds [sigma(i) | sigma(f)] per unit-half: [p, 2, 64]
            sig = [[ppB.tile([P, 2, 64], FP32, tag=f"sig{i}{h}", name=f"sig{i}{h}")
                    for h in range(2)] for i in range(2)]
            gt_ = [ppB.tile([P, P], FP32, tag=f"gt{i}", name=f"gt{i}") for i in range(2)]
            so_ = [ppB.tile([P, P], DT_C, tag=f"so{i}", name=f"so{i}") for i in range(2)]
            tct = [ppB.tile([P, P], DT_C, tag=f"tc{i}", name=f"tc{i}") for i in range(2)]
            ig_ = [ppB.tile([P, P], FP32, tag=f"ig{i}", name=f"ig{i}") for i in range(2)]
            fc_ = [ppB.tile([P, P], FP32, tag=f"fc{i}", name=f"fc{i}") for i in range(2)]

            if mm_only:
                nc.gpsimd.memset(h_sb[1][:], 0.0)
                nc.gpsimd.memset(c_sb[1][:], 0.0)

            def step(t_ap, u, tv_base):
                par, npar = u % 2, (u + 1) % 2
                # Seed both PSUM banks with zx_t via identity-matmuls, then
                # accumulate h @ U on top: activations read PSUM directly.
                nc.tensor.matmul(zb_ifg[par][:], ident_B[:], zxb[u][:, 0:384],
                                 start=True, stop=False)
                nc.tensor.matmul(zb_o[par][:], ident_B[:], zxb[u][:, 384:512],
                                 start=True, stop=False)
                # i/f/g m-tiles for unit-half 0 first, then half 1, o last
                # (separate PSUM bank): each half's c-chain starts while the
                # PE still streams the rest of the step's weights.
                m_order = (
                    [0, 1, 4, 5, 8, 9, 2, 3, 6, 7, 10, 11, 12, 13, 14, 15]
                    if not ew_only else [0, 12]
                )
                for m in m_order:
                    if m < 12:
                        tgt = zb_ifg[par][:, m * 32:(m + 1) * 32]
                    else:
                        tgt = zb_o[par][:, (m - 12) * 32:(m - 11) * 32]
                    for k in range(KC):
                        nc.tensor.matmul(
                            tgt,
                            Uk[k][:, m * P:(m + 1) * P],
                            h_sb[par][:, k * 32:(k + 1) * 32],
                            start=False,
                            stop=(k == KC - 1),
                            skip_group_check=True,
                        )
                if mm_only:
                    return
                if not half_chain:
                    nc.scalar.activation(
                        sig[par][0][:].rearrange("p g c -> p (g c)"),
                        zb_ifg[par][:, 0:128], AF.Sigmoid)
                    nc.scalar.activation(
                        sig[par][1][:].rearrange("p g c -> p (g c)"),
                        zb_ifg[par][:, 128:256], AF.Sigmoid)
                    nc.scalar.activation(gt_[par][:], zb_ifg[par][:, 256:384], AF.Tanh)
                    sigi = sig[par][0][:].rearrange("p g c -> p (g c)")
                    sigf = sig[par][1][:].rearrange("p g c -> p (g c)")
                    nc.vector.tensor_mul(fc_[par][:], sigf, c_sb[par][:])
                    nc.vector.tensor_mul(ig_[par][:], sigi, gt_[par][:])
                    nc.vector.tensor_add(c_sb[npar][:], fc_[par][:], ig_[par][:])
                    nc.scalar.activation(tct[par][:], c_sb[npar][:], AF.Tanh)
                    nc.scalar.activation(so_[par][:], zb_o[par][:], AF.Sigmoid)
                    nc.vector.tensor_mul(h_sb[npar][:], so_[par][:], tct[par][:])
                    nc.sync.dma_start(outT[ts(t_ap, P), :], h_sb[npar][:])
                    nc.sync.dma_start(
                        zxb[u][:].rearrange("p (m b) -> p m b", m=MT),
                        zxS_r[ds((tv_base + unroll) * (P // 16), P), :, u * 32:(u + 1) * 32],
                    )
                    return
                for h in range(2):
                    cs = slice(h * 64, h * 64 + 64)
                    # sigma over [i|f] of this half: psum cols i at h*64,
                    # f at 128 + h*64 - one ACT op via a 2-run AP.
                    zif = (
                        zb_ifg[par][:, 0:256]
                        .rearrange("p (g c) -> p g c", g=2)[:, :, h * 64:h * 64 + 64]
                    )
                    nc.scalar.activation(sig[par][h][:], zif, AF.Sigmoid)
                    nc.scalar.activation(
                        gt_[par][:, cs], zb_ifg[par][:, 256 + h * 64:256 + h * 64 + 64],
                        AF.Tanh,
                    )
                    nc.vector.tensor_mul(
                        fc_[par][:, cs], sig[par][h][:, 1, :], c_sb[par][:, cs]
                    )
                    nc.vector.tensor_mul(
                        ig_[par][:, cs], sig[par][h][:, 0, :], gt_[par][:, cs]
                    )
                    nc.vector.tensor_add(
                        c_sb[npar][:, cs], fc_[par][:, cs], ig_[par][:, cs]
                    )
                    nc.scalar.activation(
                        tct[par][:, cs], c_sb[npar][:, cs], AF.Tanh
                    )
                nc.scalar.activation(so_[par][:], zb_o[par][:], AF.Sigmoid)
                nc.vector.tensor_mul(h_sb[npar][:], so_[par][:], tct[par][:])
                nc.sync.dma_start(outT[ts(t_ap, P), :], h_sb[npar][:])
                nc.sync.dma_start(
                    zxb[u][:].rearrange("p (m b) -> p m b", m=MT),
                    zxS_r[ds((tv_base + unroll) * (P // 16), P), :, u * 32:(u + 1) * 32],
                )

            assert unroll == 16, "block-major zx gather needs unroll=16"
            with tc.For_i(0, s_run, unroll, staggered_reset=staggered,
                          hint_engines=hints) as tv:
                for u in range(unroll):
                    step(tv + u, u, tv)
            nc.sync.dma_start(hT_o[:], h_sb[0][:])
            nc.sync.dma_start(cT_o[:], c_sb[0][:])
    nc.compile()
    return nc


N_CORES = 2


def _make_exec(nc):
    """Jitted shard_map executor over the first N_CORES neuron devices.

    Same mechanism as bass2jax.run_bass_via_pjrt, kept local so the
    executable and device-resident inputs can be reused for timing.
    """
    bass2jax.install_neuronx_cc_hook()
    partition_name = nc.partition_id_tensor.name if nc.partition_id_tensor else None
    in_names, out_names, out_avals = [], [], []
    for alloc in nc.m.functions[0].allocations:
        if not isinstance(alloc, mybir.MemoryLocationSet):
            continue
        name = alloc.memorylocations[0].name
        if alloc.kind == "ExternalInput":
            if name != partition_name:
                in_names.append(name)
        elif alloc.kind == "ExternalOutput":
            shape = tuple(alloc.tensor_shape)
            dtype = mybir.dt.np(alloc.dtype)
            out_names.append(name)
            out_avals.append(jax.core.ShapedArray(shape, dtype))
    n_params = len(in_names)
    all_in_names = in_names + out_names
    if partition_name is not None:
        all_in_names = all_in_names + [partition_name]

    def _body(*args):
        operands = list(args)
        if partition_name is not None:
            operands.append(bass2jax.partition_id_tensor())
        outs = bass2jax._bass_exec_p.bind(
            *operands,
            out_avals=tuple(out_avals),
            in_names=tuple(all_in_names),
            out_names=tuple(out_names),
            lowering_input_output_aliases=(),
            sim_require_finite=True,
            sim_require_nnan=True,
            nc=nc,
        )
        return tuple(outs)

    devices = jax.devices()[:N_CORES]
    mesh = Mesh(np.asarray(devices), ("core",))
    nin = n_params + len(out_names)
    sharded = jax.jit(
        shard_map(
            _body,
            mesh=mesh,
            in_specs=(PartitionSpec("core"),) * nin,
            out_specs=(PartitionSpec("core"),) * len(out_names),
            check_rep=False,
        ),
        keep_unused=True,
    )
    return sharded, mesh, in_names, out_names, out_avals


def _run_spmd(in_maps):
    """Execute on cores 0..N_CORES-1, retrying through a backend reset if
    the device wedges (intermittent NRT_EXEC_UNIT_UNRECOVERABLE on first
    execution); optionally time repeated executions."""
    last_err = None
    for attempt in range(3):
        try:
            return _run_spmd_inner(in_maps)
        except Exception as e:  # noqa: BLE001 - any runtime error warrants retry
            last_err = e
            LAST_RESULT["retry_error"] = repr(e)
            try:
                jax.clear_caches()
                jax.extend.backend.clear_backends()
            except Exception:
                pass
            _CACHE.pop("exec", None)
            _CACHE.pop("nc", None)
            time.sleep(5)
    raise last_err


def _run_spmd_inner(in_maps):
    if "nc" not in _CACHE:
        _CACHE["nc"] = _build()
        _CACHE["exec"] = _make_exec(_CACHE["nc"])
    sharded, mesh, in_names, out_names, out_avals = _CACHE["exec"]
    sh = NamedSharding(mesh, PartitionSpec("core"))
    args = [
        jax.device_put(
            np.concatenate([np.asarray(m[n]) for m in in_maps], axis=0), sh
        )
        for n in in_names
    ]
    args += [
        jax.device_put(
            np.zeros((N_CORES * av.shape[0], *av.shape[1:]), av.dtype), sh
        )
        for av in out_avals
    ]
    outs = sharded(*args)
    jax.block_until_ready(outs)
    n_time = int(os.environ.get("BASS_LSTM_TIME", "0"))
    if n_time > 0:
        # Serial timing (includes per-exec axon roundtrip)...
        times = []
        for _ in range(n_time):
            t0 = time.perf_counter()
            outs2 = sharded(*args)
            jax.block_until_ready(outs2)
            times.append(time.perf_counter() - t0)
        LAST_RESULT["times_ms"] = [t * 1e3 for t in times]
        # ...and pipelined timing: dispatch a burst, block once. Device
        # executions queue back-to-back, amortizing the tunnel roundtrip.
        burst = 16
        t0 = time.perf_counter()
        pending = [sharded(*args) for _ in range(burst)]
        jax.block_until_ready(pending)
        per = (time.perf_counter() - t0) / burst
        LAST_RESULT["pipelined_ms"] = per * 1e3
        LAST_RESULT["exec_time_ns"] = int(per * 1e9)
        LAST_RESULT["timer"] = (sharded, args)
    results = []
    for c in range(N_CORES):
        results.append(
            {
                n: np.asarray(outs[i]).reshape(N_CORES, *out_avals[i].shape)[c]
                for i, n in enumerate(out_names)
            }
        )
    return results


def _pack_state(v):
    # [B, UNITS] -> [128, 128] with [p, uc*32 + b] = v[b, uc*128 + p]
    return np.ascontiguousarray(
        np.asarray(v, np.float32).reshape(B, KC, P).transpose(2, 1, 0).reshape(P, P)
    )


def _unpack_state(m):
    # inverse of _pack_state
    return np.ascontiguousarray(
        np.asarray(m, np.float32).reshape(P, KC, B).transpose(2, 1, 0).reshape(B, UNITS)
    )


def _unpack_out(o):
    # [S*128, 128] with row t*128+p, col uc*32+b  ->  [B, S, UNITS]
    return np.ascontiguousarray(
        np.asarray(o, np.float32)
        .reshape(S, P, KC, B)
        .transpose(3, 0, 2, 1)
        .reshape(B, S, UNITS)
    )


def kernel(x, fh, fc, bh, bc, emb, Wf, Uf, bf, Wb, Ub, bb):
    x = np.asarray(x)
    emb_h = np.ascontiguousarray(np.asarray(emb, np.float32).astype(NP_C))
    idx_f = np.ascontiguousarray(x.T.astype(np.int32).reshape(-1))        # [S*B] t-major
    idx_b = np.ascontiguousarray(x.T[::-1].astype(np.int32).reshape(-1))  # reversed time

    def prep(Wm, Um, bm, h0, c0, idx):
        return {
            "emb": emb_h,
            "idx": idx,
            "W": np.ascontiguousarray(np.asarray(Wm, np.float32).astype(NP_C)),
            "U": np.ascontiguousarray(np.asarray(Um, np.float32).astype(NP_C)),
            "b": np.ascontiguousarray(
                np.asarray(bm, np.float32).reshape(MT, P).T
            ),
            "h0": _pack_state(h0),
            "c0": _pack_state(c0),
        }

    in_maps = [
        prep(Wf, Uf, bf, fh, fc, idx_f),
        prep(Wb, Ub, bb, bh, bc, idx_b),
    ]
    r0, r1 = _run_spmd(in_maps)
    out_f = _unpack_out(r0["outT"])
    out_b = _unpack_out(r1["outT"])[:, ::-1, :]
    output = np.concatenate([out_f, out_b], axis=-1)
    fhT = _unpack_state(r0["hT"])
    fcT = _unpack_state(r0["cT"])
    bhT = _unpack_state(r1["hT"])
    bcT = _unpack_state(r1["cT"])
    return output, fhT, fcT, bhT, bcT


# revision 37
# speedup vs baseline: 1.8922x; 1.0533x over previous
# Bidirectional LSTM encoder (nn_Encoder) on Trainium2 via Bass/Tile.
#
# Strategy:
#   - The per-timestep recurrent matmul h @ U ([B,512]@[512,2048]) is
#     LDWEIGHTS-bandwidth-bound on the PE: its cost is independent of batch
#     (B=32 << 128), so data-parallelism over batch buys nothing for the
#     recurrence. Instead: one direction per core (core 0 forward, core 1
#     backward), full batch on each, identical SPMD program - the direction
#     lives entirely in the per-core inputs (core 1 gets time-reversed token
#     indices and the backward weights; its outputs are re-reversed on host).
#   - Fully transposed on-chip layout: z^T, h^T, c^T keep the gate/unit dim on
#     partitions, so the gate elementwise runs on all 128 lanes and the h^T
#     produced by the elementwise is directly the moving operand of the next
#     step's matmuls. No transposes inside the loop.
#   - Pre-phase per core: indirect-DMA embedding gather -> PE-transpose ->
#     zx^T = (xe @ W + b)^T staged to HBM in [t][p][m*32+b] layout so the
#     recurrence streams one contiguous [128, 512] tile per step.
#   - Recurrence: per step 64 accumulating matmuls (stationary = U tiles,
#     moving = h^T chunks, N=32), o-gate tiles last and in a separate PSUM
#     bank so the i/f/g elementwise chain overlaps the o-gate matmuls.
import os
import time

import numpy as np
import jax
import jax.extend
from jax.experimental.shard_map import shard_map
from jax.sharding import Mesh, NamedSharding, PartitionSpec

from concourse import bacc, bass2jax, mybir, tile
import concourse.bass as bass
from concourse.bass import ds, ts
from concourse.masks import make_identity

VOCAB, EMB, UNITS, B, S = 32000, 256, 512, 32, 512
H4 = 4 * UNITS            # 2048 gate width
P = 128
MT = H4 // P              # 16 m-tiles over the gate dim
KC = UNITS // P           # 4 k-chunks over the hidden dim
ECH = EMB // P            # 2 k-chunks over the embedding dim
NTOK = B * S              # 16384 tokens
GT = NTOK // P            # 128 gather tiles
NZ = 512                  # token-chunk per zx matmul (16 timesteps x 32 batch)
NB = NTOK // NZ           # 32 n-chunks
UNROLL = 16               # steps per hardware-loop iteration

FP32 = mybir.dt.float32
DT_C = mybir.dt.float16   # matmul operand dtype (weights, xe^T, h^T)
NP_C = np.float16
DT_Z = mybir.dt.float16   # zx^T HBM staging dtype
AF = mybir.ActivationFunctionType

_CACHE = {}
LAST_RESULT = {}


def _build(s_run=None, skip_pre=False, unroll=UNROLL, staggered=False, hints=(), mm_only=False, ew_only=False, no_barrier=False, pre_reps=1, half_chain=False, skip_gather=False, skip_zmm=False, h_split=True):
    # s_run/skip_pre are benchmarking knobs (shorter recurrence / no pre-phase);
    # the graded path always uses the defaults.
    if s_run is None:
        s_run = S
    nc = bacc.Bacc(
        "TRN2",
        target_bir_lowering=False,
        debug=False,
        enable_asserts=True,
        num_devices=2,
    )
    emb_in = nc.dram_tensor("emb", [VOCAB, EMB], DT_C, kind="ExternalInput")
    idx_in = nc.dram_tensor("idx", [NTOK], mybir.dt.int32, kind="ExternalInput")
    W_in = nc.dram_tensor("W", [EMB, H4], DT_C, kind="ExternalInput")
    U_in = nc.dram_tensor("U", [UNITS, H4], DT_C, kind="ExternalInput")
    b_in = nc.dram_tensor("b", [P, MT], FP32, kind="ExternalInput")
    h0_in = nc.dram_tensor("h0", [P, P], FP32, kind="ExternalInput")
    c0_in = nc.dram_tensor("c0", [P, P], FP32, kind="ExternalInput")
    outT = nc.dram_tensor("outT", [max(S, s_run) * P, P], DT_C, kind="ExternalOutput")
    hT_o = nc.dram_tensor("hT", [P, P], DT_C, kind="ExternalOutput")
    cT_o = nc.dram_tensor("cT", [P, P], FP32, kind="ExternalOutput")
    # zx^T staging: row = t*128 + p (p = unit-within-m-tile), col = m*32 + b.
    # UNROLL extra zero rows absorb the prefetch overrun of the last iteration.
    zx_kind = (
        "ExternalOutput" if os.environ.get("BASS_LSTM_DEBUG_ZX") else "Internal"
    )
    # Block-major zx staging: block (m, nb) holds zs [128, 512] contiguously,
    # so pre-phase writes run at full DMA bandwidth; the recurrence pays the
    # strided gather on its reads, hidden under the per-step matmul time.
    NBR = max(S, s_run) // 16 + 1   # +1 pad block per m-tile for prefetch overrun
    zxS = nc.dram_tensor("zxS", [MT * NBR * P, NZ], DT_Z, kind=zx_kind)

    def phase_a(tc):
        with (
            tc.tile_pool(name="ppA", bufs=1) as ppA,
            tc.tile_pool(name="wpA", bufs=4) as wpA,
            tc.tile_pool(name="psT", bufs=2, space="PSUM") as psT,
            tc.tile_pool(name="psZ", bufs=2, space="PSUM") as psZ,
        ):
            ident = ppA.tile([P, P], DT_C, tag="ident", name="ident")
            make_identity(nc, ident[:])
            Wk = [ppA.tile([P, H4], DT_C, tag=f"Wk{e}", name=f"Wk{e}") for e in range(ECH)]
            for e in range(ECH):
                nc.sync.dma_start(Wk[e][:], W_in[e * P:(e + 1) * P, :])
            b_sb = ppA.tile([P, MT], FP32, tag="b", name="b_sb")
            nc.sync.dma_start(b_sb[:], b_in[:])
            xeT = [ppA.tile([P, NTOK], DT_C, tag=f"xeT{e}", name=f"xeT{e}") for e in range(ECH)]
            for g in range(GT):
                it = wpA.tile([P, 1], mybir.dt.int32, tag="idx", name="it")
                nc.sync.dma_start(it[:], idx_in[g * P:(g + 1) * P, None])
                xg = wpA.tile([P, EMB], DT_C, tag="xe", name="xg")
                if skip_gather:
                    nc.gpsimd.memset(xg[:], 0.0)
                else:
                    nc.gpsimd.indirect_dma_start(
                        out=xg[:],
                        out_offset=None,
                        in_=emb_in[:],
                        in_offset=bass.IndirectOffsetOnAxis(ap=it[:, :1], axis=0),
                    )
                for e in range(ECH):
                    tp = psT.tile([P, P], DT_C, tag="tp", name="tp")
                    nc.tensor.transpose(
                        out=tp[:], in_=xg[:, e * P:(e + 1) * P], identity=ident[:]
                    )
                    nc.vector.tensor_copy(xeT[e][:, g * P:(g + 1) * P], tp[:])
            mn_order = (
                [(m, nb) for m in range(MT) for nb in range(NB)]
                if not no_barrier
                else [(m, nb) for nb in range(NB) for m in range(MT)]
            )
            if True:
                for m, nb in mn_order:
                    pz = psZ.tile([P, NZ], FP32, tag="pz", name="pz")
                    for e in range(ECH) if not skip_zmm else range(1):
                        nc.tensor.matmul(
                            pz[:],
                            Wk[e][:, m * P:(m + 1) * P],
                            xeT[e][:, nb * NZ:(nb + 1) * NZ],
                            start=(e == 0),
                            stop=(e == ECH - 1),
                        )
                    zs = wpA.tile([P, NZ], DT_Z, tag="zs", name="zs")
                    nc.scalar.activation(
                        zs[:, 0:256], pz[:, 0:256], AF.Identity,
                        bias=b_sb[:, m:m + 1],
                    )
                    nc.vector.tensor_scalar(
                        zs[:, 256:512], pz[:, 256:512], b_sb[:, m:m + 1], None,
                        op0=mybir.AluOpType.add,
                    )
                    nc.sync.dma_start(
                        zxS[(m * NBR + nb) * P:(m * NBR + nb + 1) * P, :], zs[:]
                    )
    with tile.TileContext(nc) as tc:
        # Phase-B persistent state is allocated and loaded up front so the
        # U-weight DMAs and state initialization overlap phase A.
        with (
            tc.tile_pool(name="ppB", bufs=1) as ppB,
            tc.tile_pool(name="psB", bufs=1, space="PSUM") as psB,
        ):
            Uk = [ppB.tile([P, H4], DT_C, tag=f"Uk{k}", name=f"Uk{k}") for k in range(KC)]
            for k in range(KC):
                nc.sync.dma_start(Uk[k][:], U_in[k * P:(k + 1) * P, :])
            h_sb = [ppB.tile([P, P], DT_C, tag=f"h{i}", name=f"h{i}") for i in range(2)]
            c_sb = [ppB.tile([P, P], FP32, tag=f"c{i}", name=f"c{i}") for i in range(2)]
            h0f = ppB.tile([P, P], FP32, tag="h0f", name="h0f")
            nc.sync.dma_start(h0f[:], h0_in[:])
            nc.vector.tensor_copy(h_sb[0][:], h0f[:])
            nc.sync.dma_start(c_sb[0][:], c0_in[:])
            if not skip_pre:
                for _ in range(pre_reps):
                    phase_a(tc)
            if not no_barrier:
                tc.strict_bb_all_engine_barrier()
            # ---------- Phase B: the 512-step recurrence --------------------
            # [r, m, c] view of the block-major staging: r = nb*128 + p.
            zxS_r = zxS[:].rearrange("(m r) c -> r m c", m=MT)
            zxb = [ppB.tile([P, NZ], DT_Z, tag=f"zxb{u}", name=f"zxb{u}") for u in range(unroll)]
            for u in range(unroll):
                nc.sync.dma_start(
                    zxb[u][:].rearrange("p (m b) -> p m b", m=MT),
                    zxS_r[0:P, :, u * 32:(u + 1) * 32],
                )
            ident_B = ppB.tile([P, P], DT_C, tag="identB", name="ident_B")
            make_identity(nc, ident_B[:])
            zb_ifg = [psB.tile([P, 384], FP32, tag=f"zbifg{i}", name=f"zbifg{i}") for i in range(2)]
            zb_o = [psB.tile([P, P], FP32, tag=f"zbo{i}", name=f"zbo{i}") for i in range(2)]
            # sig holds [sigma(i) | sigma(f)] per unit-half: [p, 2, 64]
            sig = [[ppB.tile([P, 2, 64], FP32, tag=f"sig{i}{h}", name=f"sig{i}{h}")
                    for h in range(2)] for i in range(2)]
            gt_ = [ppB.tile([P, P], FP32, tag=f"gt{i}", name=f"gt{i}") for i in range(2)]
            so_ = [ppB.tile([P, P], DT_C, tag=f"so{i}", name=f"so{i}") for i in range(2)]
            tct = [ppB.tile([P, P], DT_C, tag=f"tc{i}", name=f"tc{i}") for i in range(2)]
            ig_ = [ppB.tile([P, P], FP32, tag=f"ig{i}", name=f"ig{i}") for i in range(2)]
            fc_ = [ppB.tile([P, P], FP32, tag=f"fc{i}", name=f"fc{i}") for i in range(2)]

            if mm_only:
                nc.gpsimd.memset(h_sb[1][:], 0.0)
                nc.gpsimd.memset(c_sb[1][:], 0.0)

            def step(t_ap, u, tv_base):
                par, npar = u % 2, (u + 1) % 2
                # Seed both PSUM banks with zx_t via identity-matmuls, then
                # accumulate h @ U on top: activations read PSUM directly.
                nc.tensor.matmul(zb_ifg[par][:], ident_B[:], zxb[u][:, 0:384],
                                 start=True, stop=False)
                nc.tensor.matmul(zb_o[par][:], ident_B[:], zxb[u][:, 384:512],
                                 start=True, stop=False)
                # i/f/g m-tiles for unit-half 0 first, then half 1, o last
                # (separate PSUM bank): each half's c-chain starts while the
                # PE still streams the rest of the step's weights.
                m_order = (
                    [0, 1, 4, 5, 8, 9, 2, 3, 6, 7, 10, 11, 12, 13, 14, 15]
                    if not ew_only else [0, 12]
                )
                for m in m_order:
                    if m < 12:
                        tgt = zb_ifg[par][:, m * 32:(m + 1) * 32]
                    else:
                        tgt = zb_o[par][:, (m - 12) * 32:(m - 11) * 32]
                    for k in range(KC):
                        nc.tensor.matmul(
                            tgt,
                            Uk[k][:, m * P:(m + 1) * P],
                            h_sb[par][:, k * 32:(k + 1) * 32],
                            start=False,
                            stop=(k == KC - 1),
                            skip_group_check=True,
                        )
                if mm_only:
                    return
                if not half_chain:
                    nc.scalar.activation(
                        sig[par][0][:].rearrange("p g c -> p (g c)"),
                        zb_ifg[par][:, 0:128], AF.Sigmoid)
                    nc.scalar.activation(
                        sig[par][1][:].rearrange("p g c -> p (g c)"),
                        zb_ifg[par][:, 128:256], AF.Sigmoid)
                    nc.scalar.activation(gt_[par][:], zb_ifg[par][:, 256:384], AF.Tanh)
                    sigi = sig[par][0][:].rearrange("p g c -> p (g c)")
                    sigf = sig[par][1][:].rearrange("p g c -> p (g c)")
                    nc.vector.tensor_mul(fc_[par][:], sigf, c_sb[par][:])
                    nc.vector.tensor_mul(ig_[par][:], sigi, gt_[par][:])
                    nc.vector.tensor_add(c_sb[npar][:], fc_[par][:], ig_[par][:])
                    nc.scalar.activation(tct[par][:], c_sb[npar][:], AF.Tanh)
                    nc.scalar.activation(so_[par][:], zb_o[par][:], AF.Sigmoid)
                    if h_split:
                        # publish h in halves: next step's k0/k1 matmuls only
                        # need cols 0:64, so they start while 64:128 computes
                        nc.vector.tensor_mul(
                            h_sb[npar][:, 0:64], so_[par][:, 0:64], tct[par][:, 0:64])
                        nc.vector.tensor_mul(
                            h_sb[npar][:, 64:128], so_[par][:, 64:128], tct[par][:, 64:128])
                    else:
                        nc.vector.tensor_mul(h_sb[npar][:], so_[par][:], tct[par][:])
                    nc.sync.dma_start(outT[ts(t_ap, P), :], h_sb[npar][:])
                    nc.sync.dma_start(
                        zxb[u][:].rearrange("p (m b) -> p m b", m=MT),
                        zxS_r[ds((tv_base + unroll) * (P // 16), P), :, u * 32:(u + 1) * 32],
                    )
                    return
                for h in range(2):
                    cs = slice(h * 64, h * 64 + 64)
                    # sigma over [i|f] of this half: psum cols i at h*64,
                    # f at 128 + h*64 - one ACT op via a 2-run AP.
                    zif = (
                        zb_ifg[par][:, 0:256]
                        .rearrange("p (g c) -> p g c", g=2)[:, :, h * 64:h * 64 + 64]
                    )
                    nc.scalar.activation(sig[par][h][:], zif, AF.Sigmoid)
                    nc.scalar.activation(
                        gt_[par][:, cs], zb_ifg[par][:, 256 + h * 64:256 + h * 64 + 64],
                        AF.Tanh,
                    )
                    nc.vector.tensor_mul(
                        fc_[par][:, cs], sig[par][h][:, 1, :], c_sb[par][:, cs]
                    )
                    nc.vector.tensor_mul(
                        ig_[par][:, cs], sig[par][h][:, 0, :], gt_[par][:, cs]
                    )
                    nc.vector.tensor_add(
                        c_sb[npar][:, cs], fc_[par][:, cs], ig_[par][:, cs]
                    )
                    nc.scalar.activation(
                        tct[par][:, cs], c_sb[npar][:, cs], AF.Tanh
                    )
                nc.scalar.activation(so_[par][:], zb_o[par][:], AF.Sigmoid)
                nc.vector.tensor_mul(h_sb[npar][:], so_[par][:], tct[par][:])
                nc.sync.dma_start(outT[ts(t_ap, P), :], h_sb[npar][:])
                nc.sync.dma_start(
                    zxb[u][:].rearrange("p (m b) -> p m b", m=MT),
                    zxS_r[ds((tv_base + unroll) * (P // 16), P), :, u * 32:(u + 1) * 32],
                )

            assert unroll == 16, "block-major zx gather needs unroll=16"
            with tc.For_i(0, s_run, unroll, staggered_reset=staggered,
                          hint_engines=hints) as tv:
                for u in range(unroll):
                    step(tv + u, u, tv)
            nc.sync.dma_start(hT_o[:], h_sb[0][:])
            nc.sync.dma_start(cT_o[:], c_sb[0][:])
    nc.compile()
    return nc


N_CORES = 2


def _make_exec(nc):
    """Jitted shard_map executor over the first N_CORES neuron devices.

    Same mechanism as bass2jax.run_bass_via_pjrt, kept local so the
    executable and device-resident inputs can be reused for timing.
    """
    bass2jax.install_neuronx_cc_hook()
    partition_name = nc.partition_id_tensor.name if nc.partition_id_tensor else None
    in_names, out_names, out_avals = [], [], []
    for alloc in nc.m.functions[0].allocations:
        if not isinstance(alloc, mybir.MemoryLocationSet):
            continue
        name = alloc.memorylocations[0].name
        if alloc.kind == "ExternalInput":
            if name != partition_name:
                in_names.append(name)
        elif alloc.kind == "ExternalOutput":
            shape = tuple(alloc.tensor_shape)
            dtype = mybir.dt.np(alloc.dtype)
            out_names.append(name)
            out_avals.append(jax.core.ShapedArray(shape, dtype))
    n_params = len(in_names)
    all_in_names = in_names + out_names
    if partition_name is not None:
        all_in_names = all_in_names + [partition_name]

    def _body(*args):
        operands = list(args)
        if partition_name is not None:
            operands.append(bass2jax.partition_id_tensor())
        outs = bass2jax._bass_exec_p.bind(
            *operands,
            out_avals=tuple(out_avals),
            in_names=tuple(all_in_names),
            out_names=tuple(out_names),
            lowering_input_output_aliases=(),
            sim_require_finite=True,
            sim_require_nnan=True,
            nc=nc,
        )
        return tuple(outs)

    devices = jax.devices()[:N_CORES]
    mesh = Mesh(np.asarray(devices), ("core",))
    nin = n_params + len(out_names)
    sharded = jax.jit(
        shard_map(
            _body,
            mesh=mesh,
            in_specs=(PartitionSpec("core"),) * nin,
            out_specs=(PartitionSpec("core"),) * len(out_names),
            check_rep=False,
        ),
        keep_unused=True,
    )
    return sharded, mesh, in_names, out_names, out_avals


def _run_spmd(in_maps):
    """Execute on cores 0..N_CORES-1, retrying through a backend reset if
    the device wedges (intermittent NRT_EXEC_UNIT_UNRECOVERABLE on first
    execution); optionally time repeated executions."""
    last_err = None
    for attempt in range(3):
        try:
            return _run_spmd_inner(in_maps)
        except Exception as e:  # noqa: BLE001 - any runtime error warrants retry
            last_err = e
            LAST_RESULT["retry_error"] = repr(e)
            try:
                jax.clear_caches()
                jax.extend.backend.clear_backends()
            except Exception:
                pass
            _CACHE.pop("exec", None)
            _CACHE.pop("nc", None)
            time.sleep(5)
    raise last_err


def _run_spmd_inner(in_maps):
    if "nc" not in _CACHE:
        _CACHE["nc"] = _build()
        _CACHE["exec"] = _make_exec(_CACHE["nc"])
    sharded, mesh, in_names, out_names, out_avals = _CACHE["exec"]
    sh = NamedSharding(mesh, PartitionSpec("core"))
    args = [
        jax.device_put(
            np.concatenate([np.asarray(m[n]) for m in in_maps], axis=0), sh
        )
        for n in in_names
    ]
    args += [
        jax.device_put(
            np.zeros((N_CORES * av.shape[0], *av.shape[1:]), av.dtype), sh
        )
        for av in out_avals
    ]
    outs = sharded(*args)
    jax.block_until_ready(outs)
    n_time = int(os.environ.get("BASS_LSTM_TIME", "0"))
    if n_time > 0:
        # Serial timing (includes per-exec axon roundtrip)...
        times = []
        for _ in range(n_time):
            t0 = time.perf_counter()
            outs2 = sharded(*args)
            jax.block_until_ready(outs2)
            times.append(time.perf_counter() - t0)
        LAST_RESULT["times_ms"] = [t * 1e3 for t in times]
        # ...and pipelined timing: dispatch a burst, block once. Device
        # executions queue back-to-back, amortizing the tunnel roundtrip.
        burst = 16
        t0 = time.perf_counter()
        pending = [sharded(*args) for _ in range(burst)]
        jax.block_until_ready(pending)
        per = (time.perf_counter() - t0) / burst
        LAST_RESULT["pipelined_ms"] = per * 1e3
        LAST_RESULT["exec_time_ns"] = int(per * 1e9)
        LAST_RESULT["timer"] = (sharded, args)
    results = []
    for c in range(N_CORES):
        results.append(
            {
                n: np.asarray(outs[i]).reshape(N_CORES, *out_avals[i].shape)[c]
                for i, n in enumerate(out_names)
            }
        )
    return results


def _pack_state(v):
    # [B, UNITS] -> [128, 128] with [p, uc*32 + b] = v[b, uc*128 + p]
    return np.ascontiguousarray(
        np.asarray(v, np.float32).reshape(B, KC, P).transpose(2, 1, 0).reshape(P, P)
    )


def _unpack_state(m):
    # inverse of _pack_state
    return np.ascontiguousarray(
        np.asarray(m, np.float32).reshape(P, KC, B).transpose(2, 1, 0).reshape(B, UNITS)
    )


def _unpack_out(o):
    # [S*128, 128] with row t*128+p, col uc*32+b  ->  [B, S, UNITS]
    return np.ascontiguousarray(
        np.asarray(o, np.float32)
        .reshape(S, P, KC, B)
        .transpose(3, 0, 2, 1)
        .reshape(B, S, UNITS)
    )


def kernel(x, fh, fc, bh, bc, emb, Wf, Uf, bf, Wb, Ub, bb):
    x = np.asarray(x)
    emb_h = np.ascontiguousarray(np.asarray(emb, np.float32).astype(NP_C))
    idx_f = np.ascontiguousarray(x.T.astype(np.int32).reshape(-1))        # [S*B] t-major
    idx_b = np.ascontiguousarray(x.T[::-1].astype(np.int32).reshape(-1))  # reversed time

    def prep(Wm, Um, bm, h0, c0, idx):
        return {
            "emb": emb_h,
            "idx": idx,
            "W": np.ascontiguousarray(np.asarray(Wm, np.float32).astype(NP_C)),
            "U": np.ascontiguousarray(np.asarray(Um, np.float32).astype(NP_C)),
            "b": np.ascontiguousarray(
                np.asarray(bm, np.float32).reshape(MT, P).T
            ),
            "h0": _pack_state(h0),
            "c0": _pack_state(c0),
        }

    in_maps = [
        prep(Wf, Uf, bf, fh, fc, idx_f),
        prep(Wb, Ub, bb, bh, bc, idx_b),
    ]
    r0, r1 = _run_spmd(in_maps)
    out_f = _unpack_out(r0["outT"])
    out_b = _unpack_out(r1["outT"])[:, ::-1, :]
    output = np.concatenate([out_f, out_b], axis=-1)
    fhT = _unpack_state(r0["hT"])
    fcT = _unpack_state(r0["cT"])
    bhT = _unpack_state(r1["hT"])
    bcT = _unpack_state(r1["cT"])
    return output, fhT, fcT, bhT, bcT


# revision 39
# speedup vs baseline: 1.9246x; 1.0171x over previous
# Bidirectional LSTM encoder (nn_Encoder) on Trainium2 via Bass/Tile.
#
# Strategy:
#   - The per-timestep recurrent matmul h @ U ([B,512]@[512,2048]) is
#     LDWEIGHTS-bandwidth-bound on the PE: its cost is independent of batch
#     (B=32 << 128), so data-parallelism over batch buys nothing for the
#     recurrence. Instead: one direction per core (core 0 forward, core 1
#     backward), full batch on each, identical SPMD program - the direction
#     lives entirely in the per-core inputs (core 1 gets time-reversed token
#     indices and the backward weights; its outputs are re-reversed on host).
#   - Fully transposed on-chip layout: z^T, h^T, c^T keep the gate/unit dim on
#     partitions, so the gate elementwise runs on all 128 lanes and the h^T
#     produced by the elementwise is directly the moving operand of the next
#     step's matmuls. No transposes inside the loop.
#   - Pre-phase per core: indirect-DMA embedding gather -> PE-transpose ->
#     zx^T = (xe @ W + b)^T staged to HBM in [t][p][m*32+b] layout so the
#     recurrence streams one contiguous [128, 512] tile per step.
#   - Recurrence: per step 64 accumulating matmuls (stationary = U tiles,
#     moving = h^T chunks, N=32), o-gate tiles last and in a separate PSUM
#     bank so the i/f/g elementwise chain overlaps the o-gate matmuls.
import os
import time

import numpy as np
import jax
import jax.extend
from jax.experimental.shard_map import shard_map
from jax.sharding import Mesh, NamedSharding, PartitionSpec

from concourse import bacc, bass2jax, mybir, tile
import concourse.bass as bass
from concourse.bass import ds, ts
from concourse.masks import make_identity

VOCAB, EMB, UNITS, B, S = 32000, 256, 512, 32, 512
H4 = 4 * UNITS            # 2048 gate width
P = 128
MT = H4 // P              # 16 m-tiles over the gate dim
KC = UNITS // P           # 4 k-chunks over the hidden dim
ECH = EMB // P            # 2 k-chunks over the embedding dim
NTOK = B * S              # 16384 tokens
GT = NTOK // P            # 128 gather tiles
NZ = 512                  # token-chunk per zx matmul (16 timesteps x 32 batch)
NB = NTOK // NZ           # 32 n-chunks
UNROLL = 16               # steps per hardware-loop iteration

FP32 = mybir.dt.float32
DT_C = mybir.dt.float16   # matmul operand dtype (weights, xe^T, h^T)
NP_C = np.float16
DT_Z = mybir.dt.float16   # zx^T HBM staging dtype
AF = mybir.ActivationFunctionType

_CACHE = {}
LAST_RESULT = {}


def _build(s_run=None, skip_pre=False, unroll=UNROLL, staggered=False, hints=(), mm_only=False, ew_only=False, no_barrier=False, pre_reps=1, half_chain=False, skip_gather=False, skip_zmm=False, h_split=True, g_first=True):
    # s_run/skip_pre are benchmarking knobs (shorter recurrence / no pre-phase);
    # the graded path always uses the defaults.
    if s_run is None:
        s_run = S
    nc = bacc.Bacc(
        "TRN2",
        target_bir_lowering=False,
        debug=False,
        enable_asserts=True,
        num_devices=2,
    )
    emb_in = nc.dram_tensor("emb", [VOCAB, EMB], DT_C, kind="ExternalInput")
    idx_in = nc.dram_tensor("idx", [NTOK], mybir.dt.int32, kind="ExternalInput")
    W_in = nc.dram_tensor("W", [EMB, H4], DT_C, kind="ExternalInput")
    U_in = nc.dram_tensor("U", [UNITS, H4], DT_C, kind="ExternalInput")
    b_in = nc.dram_tensor("b", [P, MT], FP32, kind="ExternalInput")
    h0_in = nc.dram_tensor("h0", [P, P], FP32, kind="ExternalInput")
    c0_in = nc.dram_tensor("c0", [P, P], FP32, kind="ExternalInput")
    outT = nc.dram_tensor("outT", [max(S, s_run) * P, P], DT_C, kind="ExternalOutput")
    hT_o = nc.dram_tensor("hT", [P, P], DT_C, kind="ExternalOutput")
    cT_o = nc.dram_tensor("cT", [P, P], FP32, kind="ExternalOutput")
    # zx^T staging: row = t*128 + p (p = unit-within-m-tile), col = m*32 + b.
    # UNROLL extra zero rows absorb the prefetch overrun of the last iteration.
    zx_kind = (
        "ExternalOutput" if os.environ.get("BASS_LSTM_DEBUG_ZX") else "Internal"
    )
    # Block-major zx staging: block (m, nb) holds zs [128, 512] contiguously,
    # so pre-phase writes run at full DMA bandwidth; the recurrence pays the
    # strided gather on its reads, hidden under the per-step matmul time.
    NBR = max(S, s_run) // 16 + 1   # +1 pad block per m-tile for prefetch overrun
    zxS = nc.dram_tensor("zxS", [MT * NBR * P, NZ], DT_Z, kind=zx_kind)

    def phase_a(tc):
        with (
            tc.tile_pool(name="ppA", bufs=1) as ppA,
            tc.tile_pool(name="wpA", bufs=4) as wpA,
            tc.tile_pool(name="psT", bufs=2, space="PSUM") as psT,
            tc.tile_pool(name="psZ", bufs=2, space="PSUM") as psZ,
        ):
            ident = ppA.tile([P, P], DT_C, tag="ident", name="ident")
            make_identity(nc, ident[:])
            Wk = [ppA.tile([P, H4], DT_C, tag=f"Wk{e}", name=f"Wk{e}") for e in range(ECH)]
            for e in range(ECH):
                nc.sync.dma_start(Wk[e][:], W_in[e * P:(e + 1) * P, :])
            b_sb = ppA.tile([P, MT], FP32, tag="b", name="b_sb")
            nc.sync.dma_start(b_sb[:], b_in[:])
            xeT = [ppA.tile([P, NTOK], DT_C, tag=f"xeT{e}", name=f"xeT{e}") for e in range(ECH)]
            for g in range(GT):
                it = wpA.tile([P, 1], mybir.dt.int32, tag="idx", name="it")
                nc.sync.dma_start(it[:], idx_in[g * P:(g + 1) * P, None])
                xg = wpA.tile([P, EMB], DT_C, tag="xe", name="xg")
                if skip_gather:
                    nc.gpsimd.memset(xg[:], 0.0)
                else:
                    nc.gpsimd.indirect_dma_start(
                        out=xg[:],
                        out_offset=None,
                        in_=emb_in[:],
                        in_offset=bass.IndirectOffsetOnAxis(ap=it[:, :1], axis=0),
                    )
                for e in range(ECH):
                    tp = psT.tile([P, P], DT_C, tag="tp", name="tp")
                    nc.tensor.transpose(
                        out=tp[:], in_=xg[:, e * P:(e + 1) * P], identity=ident[:]
                    )
                    nc.vector.tensor_copy(xeT[e][:, g * P:(g + 1) * P], tp[:])
            mn_order = (
                [(m, nb) for m in range(MT) for nb in range(NB)]
                if not no_barrier
                else [(m, nb) for nb in range(NB) for m in range(MT)]
            )
            if True:
                for m, nb in mn_order:
                    pz = psZ.tile([P, NZ], FP32, tag="pz", name="pz")
                    for e in range(ECH) if not skip_zmm else range(1):
                        nc.tensor.matmul(
                            pz[:],
                            Wk[e][:, m * P:(m + 1) * P],
                            xeT[e][:, nb * NZ:(nb + 1) * NZ],
                            start=(e == 0),
                            stop=(e == ECH - 1),
                        )
                    zs = wpA.tile([P, NZ], DT_Z, tag="zs", name="zs")
                    nc.scalar.activation(
                        zs[:, 0:256], pz[:, 0:256], AF.Identity,
                        bias=b_sb[:, m:m + 1],
                    )
                    nc.vector.tensor_scalar(
                        zs[:, 256:512], pz[:, 256:512], b_sb[:, m:m + 1], None,
                        op0=mybir.AluOpType.add,
                    )
                    nc.sync.dma_start(
                        zxS[(m * NBR + nb) * P:(m * NBR + nb + 1) * P, :], zs[:]
                    )
    with tile.TileContext(nc) as tc:
        # Phase-B persistent state is allocated and loaded up front so the
        # U-weight DMAs and state initialization overlap phase A.
        with (
            tc.tile_pool(name="ppB", bufs=1) as ppB,
            tc.tile_pool(name="psB", bufs=1, space="PSUM") as psB,
        ):
            Uk = [ppB.tile([P, H4], DT_C, tag=f"Uk{k}", name=f"Uk{k}") for k in range(KC)]
            for k in range(KC):
                nc.sync.dma_start(Uk[k][:], U_in[k * P:(k + 1) * P, :])
            h_sb = [ppB.tile([P, P], DT_C, tag=f"h{i}", name=f"h{i}") for i in range(2)]
            c_sb = [ppB.tile([P, P], FP32, tag=f"c{i}", name=f"c{i}") for i in range(2)]
            h0f = ppB.tile([P, P], FP32, tag="h0f", name="h0f")
            nc.sync.dma_start(h0f[:], h0_in[:])
            nc.vector.tensor_copy(h_sb[0][:], h0f[:])
            nc.sync.dma_start(c_sb[0][:], c0_in[:])
            if not skip_pre:
                for _ in range(pre_reps):
                    phase_a(tc)
            if not no_barrier:
                tc.strict_bb_all_engine_barrier()
            # ---------- Phase B: the 512-step recurrence --------------------
            # [r, m, c] view of the block-major staging: r = nb*128 + p.
            zxS_r = zxS[:].rearrange("(m r) c -> r m c", m=MT)
            zxb = [ppB.tile([P, NZ], DT_Z, tag=f"zxb{u}", name=f"zxb{u}") for u in range(unroll)]
            for u in range(unroll):
                nc.sync.dma_start(
                    zxb[u][:].rearrange("p (m b) -> p m b", m=MT),
                    zxS_r[0:P, :, u * 32:(u + 1) * 32],
                )
            ident_B = ppB.tile([P, P], DT_C, tag="identB", name="ident_B")
            make_identity(nc, ident_B[:])
            zb_ifg = [psB.tile([P, 384], FP32, tag=f"zbifg{i}", name=f"zbifg{i}") for i in range(2)]
            zb_o = [psB.tile([P, P], FP32, tag=f"zbo{i}", name=f"zbo{i}") for i in range(2)]
            # sig holds [sigma(i) | sigma(f)] per unit-half: [p, 2, 64]
            sig = [[ppB.tile([P, 2, 64], FP32, tag=f"sig{i}{h}", name=f"sig{i}{h}")
                    for h in range(2)] for i in range(2)]
            gt_ = [ppB.tile([P, P], FP32, tag=f"gt{i}", name=f"gt{i}") for i in range(2)]
            so_ = [ppB.tile([P, P], DT_C, tag=f"so{i}", name=f"so{i}") for i in range(2)]
            tct = [ppB.tile([P, P], DT_C, tag=f"tc{i}", name=f"tc{i}") for i in range(2)]
            ig_ = [ppB.tile([P, P], FP32, tag=f"ig{i}", name=f"ig{i}") for i in range(2)]
            fc_ = [ppB.tile([P, P], FP32, tag=f"fc{i}", name=f"fc{i}") for i in range(2)]

            if mm_only:
                nc.gpsimd.memset(h_sb[1][:], 0.0)
                nc.gpsimd.memset(c_sb[1][:], 0.0)

            def step(t_ap, u, tv_base):
                par, npar = u % 2, (u + 1) % 2
                # Seed both PSUM banks with zx_t via identity-matmuls, then
                # accumulate h @ U on top: activations read PSUM directly.
                nc.tensor.matmul(zb_ifg[par][:], ident_B[:], zxb[u][:, 0:384],
                                 start=True, stop=False)
                nc.tensor.matmul(zb_o[par][:], ident_B[:], zxb[u][:, 384:512],
                                 start=True, stop=False)
                # i/f/g m-tiles for unit-half 0 first, then half 1, o last
                # (separate PSUM bank): each half's c-chain starts while the
                # PE still streams the rest of the step's weights.
                if ew_only:
                    m_order = [0, 12]
                elif g_first:
                    # g, i, f, o: each gate's activation fires as soon as its
                    # own matmuls finish; the c-chain overlaps the f/o streams
                    m_order = [8, 9, 10, 11, 0, 1, 2, 3, 4, 5, 6, 7, 12, 13, 14, 15]
                else:
                    m_order = [0, 1, 4, 5, 8, 9, 2, 3, 6, 7, 10, 11, 12, 13, 14, 15]
                for m in m_order:
                    if m < 12:
                        tgt = zb_ifg[par][:, m * 32:(m + 1) * 32]
                    else:
                        tgt = zb_o[par][:, (m - 12) * 32:(m - 11) * 32]
                    for k in range(KC):
                        nc.tensor.matmul(
                            tgt,
                            Uk[k][:, m * P:(m + 1) * P],
                            h_sb[par][:, k * 32:(k + 1) * 32],
                            start=False,
                            stop=(k == KC - 1),
                            skip_group_check=True,
                        )
                if mm_only:
                    return
                if not half_chain:
                    if g_first:
                        nc.scalar.activation(gt_[par][:], zb_ifg[par][:, 256:384], AF.Tanh)
                    nc.scalar.activation(
                        sig[par][0][:].rearrange("p g c -> p (g c)"),
                        zb_ifg[par][:, 0:128], AF.Sigmoid)
                    nc.scalar.activation(
                        sig[par][1][:].rearrange("p g c -> p (g c)"),
                        zb_ifg[par][:, 128:256], AF.Sigmoid)
                    if not g_first:
                        nc.scalar.activation(gt_[par][:], zb_ifg[par][:, 256:384], AF.Tanh)
                    sigi = sig[par][0][:].rearrange("p g c -> p (g c)")
                    sigf = sig[par][1][:].rearrange("p g c -> p (g c)")
                    if g_first:
                        nc.vector.tensor_mul(ig_[par][:], sigi, gt_[par][:])
                        nc.vector.tensor_mul(fc_[par][:], sigf, c_sb[par][:])
                    else:
                        nc.vector.tensor_mul(fc_[par][:], sigf, c_sb[par][:])
                        nc.vector.tensor_mul(ig_[par][:], sigi, gt_[par][:])
                    nc.vector.tensor_add(c_sb[npar][:], fc_[par][:], ig_[par][:])
                    nc.scalar.activation(tct[par][:], c_sb[npar][:], AF.Tanh)
                    nc.scalar.activation(so_[par][:], zb_o[par][:], AF.Sigmoid)
                    if h_split:
                        # publish h in halves: next step's k0/k1 matmuls only
                        # need cols 0:64, so they start while 64:128 computes
                        nc.vector.tensor_mul(
                            h_sb[npar][:, 0:64], so_[par][:, 0:64], tct[par][:, 0:64])
                        nc.vector.tensor_mul(
                            h_sb[npar][:, 64:128], so_[par][:, 64:128], tct[par][:, 64:128])
                    else:
                        nc.vector.tensor_mul(h_sb[npar][:], so_[par][:], tct[par][:])
                    nc.sync.dma_start(outT[ts(t_ap, P), :], h_sb[npar][:])
                    nc.sync.dma_start(
                        zxb[u][:].rearrange("p (m b) -> p m b", m=MT),
                        zxS_r[ds((tv_base + unroll) * (P // 16), P), :, u * 32:(u + 1) * 32],
                    )
                    return
                for h in range(2):
                    cs = slice(h * 64, h * 64 + 64)
                    # sigma over [i|f] of this half: psum cols i at h*64,
                    # f at 128 + h*64 - one ACT op via a 2-run AP.
                    zif = (
                        zb_ifg[par][:, 0:256]
                        .rearrange("p (g c) -> p g c", g=2)[:, :, h * 64:h * 64 + 64]
                    )
                    nc.scalar.activation(sig[par][h][:], zif, AF.Sigmoid)
                    nc.scalar.activation(
                        gt_[par][:, cs], zb_ifg[par][:, 256 + h * 64:256 + h * 64 + 64],
                        AF.Tanh,
                    )
                    nc.vector.tensor_mul(
                        fc_[par][:, cs], sig[par][h][:, 1, :], c_sb[par][:, cs]
                    )
                    nc.vector.tensor_mul(
                        ig_[par][:, cs], sig[par][h][:, 0, :], gt_[par][:, cs]
                    )
                    nc.vector.tensor_add(
                        c_sb[npar][:, cs], fc_[par][:, cs], ig_[par][:, cs]
                    )
                    nc.scalar.activation(
                        tct[par][:, cs], c_sb[npar][:, cs], AF.Tanh
                    )
                nc.scalar.activation(so_[par][:], zb_o[par][:], AF.Sigmoid)
                nc.vector.tensor_mul(h_sb[npar][:], so_[par][:], tct[par][:])
                nc.sync.dma_start(outT[ts(t_ap, P), :], h_sb[npar][:])
                nc.sync.dma_start(
                    zxb[u][:].rearrange("p (m b) -> p m b", m=MT),
                    zxS_r[ds((tv_base + unroll) * (P // 16), P), :, u * 32:(u + 1) * 32],
                )

            assert unroll == 16, "block-major zx gather needs unroll=16"
            with tc.For_i(0, s_run, unroll, staggered_reset=staggered,
                          hint_engines=hints) as tv:
                for u in range(unroll):
                    step(tv + u, u, tv)
            nc.sync.dma_start(hT_o[:], h_sb[0][:])
            nc.sync.dma_start(cT_o[:], c_sb[0][:])
    nc.compile()
    return nc


N_CORES = 2


def _make_exec(nc):
    """Jitted shard_map executor over the first N_CORES neuron devices.

    Same mechanism as bass2jax.run_bass_via_pjrt, kept local so the
    executable and device-resident inputs can be reused for timing.
    """
    bass2jax.install_neuronx_cc_hook()
    partition_name = nc.partition_id_tensor.name if nc.partition_id_tensor else None
    in_names, out_names, out_avals = [], [], []
    for alloc in nc.m.functions[0].allocations:
        if not isinstance(alloc, mybir.MemoryLocationSet):
            continue
        name = alloc.memorylocations[0].name
        if alloc.kind == "ExternalInput":
            if name != partition_name:
                in_names.append(name)
        elif alloc.kind == "ExternalOutput":
            shape = tuple(alloc.tensor_shape)
            dtype = mybir.dt.np(alloc.dtype)
            out_names.append(name)
            out_avals.append(jax.core.ShapedArray(shape, dtype))
    n_params = len(in_names)
    all_in_names = in_names + out_names
    if partition_name is not None:
        all_in_names = all_in_names + [partition_name]

    def _body(*args):
        operands = list(args)
        if partition_name is not None:
            operands.append(bass2jax.partition_id_tensor())
        outs = bass2jax._bass_exec_p.bind(
            *operands,
            out_avals=tuple(out_avals),
            in_names=tuple(all_in_names),
            out_names=tuple(out_names),
            lowering_input_output_aliases=(),
            sim_require_finite=True,
            sim_require_nnan=True,
            nc=nc,
        )
        return tuple(outs)

    devices = jax.devices()[:N_CORES]
    mesh = Mesh(np.asarray(devices), ("core",))
    nin = n_params + len(out_names)
    sharded = jax.jit(
        shard_map(
            _body,
            mesh=mesh,
            in_specs=(PartitionSpec("core"),) * nin,
            out_specs=(PartitionSpec("core"),) * len(out_names),
            check_rep=False,
        ),
        keep_unused=True,
    )
    return sharded, mesh, in_names, out_names, out_avals


def _run_spmd(in_maps):
    """Execute on cores 0..N_CORES-1, retrying through a backend reset if
    the device wedges (intermittent NRT_EXEC_UNIT_UNRECOVERABLE on first
    execution); optionally time repeated executions."""
    last_err = None
    for attempt in range(3):
        try:
            return _run_spmd_inner(in_maps)
        except Exception as e:  # noqa: BLE001 - any runtime error warrants retry
            last_err = e
            LAST_RESULT["retry_error"] = repr(e)
            try:
                jax.clear_caches()
                jax.extend.backend.clear_backends()
            except Exception:
                pass
            _CACHE.pop("exec", None)
            _CACHE.pop("nc", None)
            time.sleep(5)
    raise last_err


def _run_spmd_inner(in_maps):
    if "nc" not in _CACHE:
        _CACHE["nc"] = _build()
        _CACHE["exec"] = _make_exec(_CACHE["nc"])
    sharded, mesh, in_names, out_names, out_avals = _CACHE["exec"]
    sh = NamedSharding(mesh, PartitionSpec("core"))
    args = [
        jax.device_put(
            np.concatenate([np.asarray(m[n]) for m in in_maps], axis=0), sh
        )
        for n in in_names
    ]
    args += [
        jax.device_put(
            np.zeros((N_CORES * av.shape[0], *av.shape[1:]), av.dtype), sh
        )
        for av in out_avals
    ]
    outs = sharded(*args)
    jax.block_until_ready(outs)
    n_time = int(os.environ.get("BASS_LSTM_TIME", "0"))
    if n_time > 0:
        # Serial timing (includes per-exec axon roundtrip)...
        times = []
        for _ in range(n_time):
            t0 = time.perf_counter()
            outs2 = sharded(*args)
            jax.block_until_ready(outs2)
            times.append(time.perf_counter() - t0)
        LAST_RESULT["times_ms"] = [t * 1e3 for t in times]
        # ...and pipelined timing: dispatch a burst, block once. Device
        # executions queue back-to-back, amortizing the tunnel roundtrip.
        burst = 16
        t0 = time.perf_counter()
        pending = [sharded(*args) for _ in range(burst)]
        jax.block_until_ready(pending)
        per = (time.perf_counter() - t0) / burst
        LAST_RESULT["pipelined_ms"] = per * 1e3
        LAST_RESULT["exec_time_ns"] = int(per * 1e9)
        LAST_RESULT["timer"] = (sharded, args)
    results = []
    for c in range(N_CORES):
        results.append(
            {
                n: np.asarray(outs[i]).reshape(N_CORES, *out_avals[i].shape)[c]
                for i, n in enumerate(out_names)
            }
        )
    return results


def _pack_state(v):
    # [B, UNITS] -> [128, 128] with [p, uc*32 + b] = v[b, uc*128 + p]
    return np.ascontiguousarray(
        np.asarray(v, np.float32).reshape(B, KC, P).transpose(2, 1, 0).reshape(P, P)
    )


def _unpack_state(m):
    # inverse of _pack_state
    return np.ascontiguousarray(
        np.asarray(m, np.float32).reshape(P, KC, B).transpose(2, 1, 0).reshape(B, UNITS)
    )


def _unpack_out(o):
    # [S*128, 128] with row t*128+p, col uc*32+b  ->  [B, S, UNITS]
    return np.ascontiguousarray(
        np.asarray(o, np.float32)
        .reshape(S, P, KC, B)
        .transpose(3, 0, 2, 1)
        .reshape(B, S, UNITS)
    )


def kernel(x, fh, fc, bh, bc, emb, Wf, Uf, bf, Wb, Ub, bb):
    x = np.asarray(x)
    emb_h = np.ascontiguousarray(np.asarray(emb, np.float32).astype(NP_C))
    idx_f = np.ascontiguousarray(x.T.astype(np.int32).reshape(-1))        # [S*B] t-major
    idx_b = np.ascontiguousarray(x.T[::-1].astype(np.int32).reshape(-1))  # reversed time

    def prep(Wm, Um, bm, h0, c0, idx):
        return {
            "emb": emb_h,
            "idx": idx,
            "W": np.ascontiguousarray(np.asarray(Wm, np.float32).astype(NP_C)),
            "U": np.ascontiguousarray(np.asarray(Um, np.float32).astype(NP_C)),
            "b": np.ascontiguousarray(
                np.asarray(bm, np.float32).reshape(MT, P).T
            ),
            "h0": _pack_state(h0),
            "c0": _pack_state(c0),
        }

    in_maps = [
        prep(Wf, Uf, bf, fh, fc, idx_f),
        prep(Wb, Ub, bb, bh, bc, idx_b),
    ]
    r0, r1 = _run_spmd(in_maps)
    out_f = _unpack_out(r0["outT"])
    out_b = _unpack_out(r1["outT"])[:, ::-1, :]
    output = np.concatenate([out_f, out_b], axis=-1)
    fhT = _unpack_state(r0["hT"])
    fcT = _unpack_state(r0["cT"])
    bhT = _unpack_state(r1["hT"])
    bcT = _unpack_state(r1["cT"])
    return output, fhT, fcT, bhT, bcT
